# revision 46
# baseline (speedup 1.0000x reference)
# Trainium2 Bass kernel for DST_Decoder.
#
# Math reformulation (exact):
#   h  = relu(x @ w1 + b1);  p = h @ w2 + b2                  (pointwise MLP)
#   dx_t = p_t - p_{t-1} (p_{-1}=0);  praw_t = p_t + p_{t-1} = 2*m_t
#   S1_t = p_t;  S2_t = sum_{s<=t} m_s (x) dx_s               (Chen identity)
#   z_t  = cumsum_t[ vec(praw (x) dx) @ (0.5*W1_sig) + dx @ W1_s1 ] + bb1
#   out  = relu(z) @ W2 + bb2
# i.e. contract each timestep's rank-1 outer-product update with W1 FIRST,
# then a cheap 64-wide cumulative scan.  The 0.5 midpoint factor is folded
# into W1_sig on the host so praw needs only an add.
#
# Layout: features on SBUF partitions, time on the free axis; x pre-transposed
# bf16 from the host.  The outer-product tensor O^T (1024, t) is built k-tile
# by k-tile: PE broadcasts rows of praw^T to 128 partitions via a 0/1
# selection matrix (E_r @ praw^T -> 2-bank PSUM tile), then the product with
# a 4x-stacked bf16 copy of dx^T is formed on DVE / gpsimd / (Scalar copy +
# DVE 2x-bf16) split across k-tiles for engine balance.  All matmul operands
# are bf16 and every matmul is padded to contraction dim K=128; pad rows hold
# zero weights and rhs pad rows are zeroed ON-CHIP once per rotating buffer
# (memset is column-bound, so pads live in narrow per-batch/per-chunk tiles).
# The 64-wide scan reads the contraction PSUM accumulator directly.  All x
# batches stream in up front and every pointwise MLP runs before the first
# outer-product block, so mid-pipeline DMA is just the dx 4x-stacking and the
# (bf16) output stores.  The W1 contraction is interleaved into the E-select
# stream (lag 2) so PSUM mb tiles drain in time; the next batch's praw and
# the pending store are injected into the phase3 stream where they fill
# dependency-wait holes; the last batch drains at 256-column granularity.
# Sharding: data-parallel over batch, 4 batches per core, weights replicated.

import os
import sys

import numpy as np

for _p in ("/opt/trn_rl_repo",):
    if _p not in sys.path and os.path.isdir(_p):
        sys.path.append(_p)

from concourse import bacc, tile
from concourse import bass_utils
import concourse.mybir as mybir

F32 = mybir.dt.float32
BF16 = mybir.dt.bfloat16

N_CORES = 8
B, L, DIN = 32, 1024, 256
C, HID, DOUT = 32, 64, 128
B_CORE = B // N_CORES                 # 4 batches per core
T = B_CORE * L                        # 4096 time positions per core
KT = (C * C) // 128                   # 8 k-tiles of the outer-product block
ACT_ROUTE = (1, 3, 5, 7)              # k-tiles multiplied on DVE after ACT copy
GPS_ROUTE = ()                        # gpsimd cannot read PSUM mb tiles

# packed const layout (columns in cw)
CW_W1 = 0                             # 2*HID
CW_W2 = 128                           # C
CW_W1M = 160                          # KT*HID
CW_W1DX = 672                         # HID
CW_E = 736                            # KT*128
CW_W2B = 1760                         # DOUT
CW_COLS = 1888
CW_P1 = 160                           # phase1 needs only the first 160 cols

TRACE = False
LAST_EXEC_NS = None
LAST_PROFILE = None
LAST_TRACE_PATH = None


def build_nc(t_total=T, seq_len=L, chunk=512):
    n_batch = t_total // seq_len      # 4 batches
    cpb = seq_len // chunk            # 2 chunks per batch

    nc = bacc.Bacc(trn_type="TRN2", target_bir_lowering=False, debug=False)

    xTb = nc.dram_tensor("xTb", (n_batch, 128, 2, seq_len), BF16,
                         kind="ExternalInput").ap()
    cw = nc.dram_tensor("cw", (128, CW_COLS), BF16, kind="ExternalInput").ap()
    cb = nc.dram_tensor("cb", (128, 4), F32, kind="ExternalInput").ap()
    out = nc.dram_tensor("out", (DOUT, t_total), BF16,
                         kind="ExternalOutput").ap()

    RELU = mybir.ActivationFunctionType.Relu
    ADD = mybir.AluOpType.add
    BYP = mybir.AluOpType.bypass

    with tile.TileContext(nc) as tc:
        with (
            tc.tile_pool(name="consts", bufs=1) as cpool,
            tc.tile_pool(name="persist", bufs=1) as ppool,
            tc.tile_pool(name="xin", bufs=4) as xpool,
            tc.tile_pool(name="mbs", bufs=2) as mpool,
            tc.tile_pool(name="rhs", bufs=8) as rpool,
            tc.tile_pool(name="zbuf", bufs=2) as zpool,
            tc.tile_pool(name="obuf", bufs=6) as opool,
            tc.tile_pool(name="ps1", bufs=2, space="PSUM") as ps1,
            tc.tile_pool(name="psu", bufs=2, space="PSUM") as psu,
            tc.tile_pool(name="psm", bufs=2, space="PSUM") as psm,
        ):
            # ---- input DMAs: everything streams in up front; batch-0 x is
            # split 4 ways so the very first matmul gates on 128KB ----------
            xts = {}

            def issue_x(b, split=False):
                xt = xpool.tile([128, 2, seq_len], BF16, tag="xt")
                if split:
                    nc.sync.dma_start(out=xt[:, 0, 0:chunk],
                                      in_=xTb[b][:, 0, 0:chunk])
                    return xt
                nc.sync.dma_start(out=xt[:], in_=xTb[b])
                xts[b] = xt

            xt0 = issue_x(0, split=True)
            cw_sb = cpool.tile([128, CW_COLS], BF16, tag="cw")
            nc.sync.dma_start(out=cw_sb[:, 0:CW_P1], in_=cw[:, 0:CW_P1])
            nc.sync.dma_start(out=xt0[:, 1, 0:chunk], in_=xTb[0][:, 1, 0:chunk])
            nc.sync.dma_start(out=xt0[:, 0, chunk:seq_len],
                              in_=xTb[0][:, 0, chunk:seq_len])
            nc.sync.dma_start(out=xt0[:, 1, chunk:seq_len],
                              in_=xTb[0][:, 1, chunk:seq_len])
            xts[0] = xt0
            cb_sb = cpool.tile([128, 4], F32, tag="cb")
            nc.sync.dma_start(out=cb_sb[:], in_=cb)
            nc.sync.dma_start(out=cw_sb[:, CW_P1:CW_COLS],
                              in_=cw[:, CW_P1:CW_COLS])
            for b in range(1, n_batch):
                issue_x(b)
            b1_sb = cb_sb[0:HID, 0:1]
            b2_sb = cb_sb[0:C, 1:2]
            bb1_sb = cb_sb[0:HID, 2:3]
            bb2_sb = cb_sb[:, 3:4]

            # ---- persistent activations -----------------------------------
            pT = ppool.tile([C, t_total], F32, tag="pT")
            dx4 = ppool.tile([128, t_total], BF16, tag="dx4")
            hsbP = ppool.tile([128, 2, chunk], BF16, tag="hsbP")
            zsc = ppool.tile([HID, chunk], BF16, tag="zsc")
            # praw tiles rotate per batch, aT tiles per chunk; their pad rows
            # (zero weights in the matmuls, just need finite values) are
            # zeroed once per buffer here instead of per batch.
            mTrs = [ppool.tile([128, seq_len], BF16, tag=f"mTr{i}",
                               name=f"mTr{i}") for i in range(3)]
            aTs = [ppool.tile([128, chunk], BF16, tag=f"aTc{i}",
                              name=f"aT{i}") for i in range(4)]

            # small pads on DVE (fast, nothing queued behind them yet)
            for q in range(2):
                nc.vector.memset(hsbP[HID + 32 * q:HID + 32 * (q + 1), 0, :], 0.0)
                nc.vector.memset(hsbP[HID + 32 * q:HID + 32 * (q + 1), 1, :], 0.0)
            nc.vector.memset(zsc[:], 0.0)
            # batch-0 praw pads on gpsimd ahead of batch-0's dx diff
            for q in range(3):
                nc.gpsimd.memset(mTrs[0][C + 32 * q:C + 32 * (q + 1), :], 0.0)

            def phase1(b):
                # pointwise MLP for both chunks of batch b (x prefetched).
                # relu+bias on DVE: phase1 runs only in the up-front window
                # where DVE is idle, and the scalar queue (ACT table load +
                # bias adds) was pacing the whole chain.
                xt = xts.pop(b)
                for ci in range(cpb):
                    hps = ps1.tile([HID, chunk], F32, tag="ps1t")
                    for k in range(2):
                        nc.tensor.matmul(hps[:],
                                         cw_sb[:, CW_W1 + k * HID:
                                               CW_W1 + (k + 1) * HID],
                                         xt[:, k, ci * chunk:(ci + 1) * chunk],
                                         start=(k == 0), stop=(k == 1))
                    nc.scalar.activation(hsbP[0:HID, ci, :], hps[:], RELU,
                                         bias=b1_sb, scale=1.0)
                for ci in range(cpb):
                    cs = slice(b * seq_len + ci * chunk,
                               b * seq_len + (ci + 1) * chunk)
                    pps = ps1.tile([C, chunk], F32, tag="ps1t")
                    nc.tensor.matmul(pps[:], cw_sb[:, CW_W2:CW_W2 + C],
                                     hsbP[:, ci, :], start=True, stop=True)
                    nc.scalar.add(pT[:, cs], pps[:], b2_sb)

            def phase2_praw(b, eng=None):
                # praw (feeds the E-select broadcast); during iterations
                # gpsimd is otherwise idle, so it runs there by default --
                # only batch 0's (on the critical path to the first E-select)
                # goes on DVE.
                eng = eng or nc.gpsimd
                t0 = b * seq_len
                mTr = mTrs[b % 3]
                eng.tensor_copy(mTr[0:C, 0:1], pT[:, t0:t0 + 1])
                eng.tensor_add(
                    mTr[0:C, 1:seq_len],
                    pT[:, t0 + 1:t0 + seq_len],
                    pT[:, t0:t0 + seq_len - 1],
                )

            def phase2_dx(b):
                # dx diff straight to bf16 dx4 rows 0:32 on gpsimd (split per
                # chunk), then 4x partition-stacking via two log-doubling
                # SBUF-to-SBUF DMAs
                t0 = b * seq_len
                bs = slice(t0, t0 + seq_len)
                nc.gpsimd.tensor_copy(dx4[0:C, t0:t0 + 1], pT[:, t0:t0 + 1])
                nc.gpsimd.tensor_sub(
                    dx4[0:C, t0 + 1:t0 + chunk],
                    pT[:, t0 + 1:t0 + chunk],
                    pT[:, t0:t0 + chunk - 1],
                )
                nc.gpsimd.tensor_sub(
                    dx4[0:C, t0 + chunk:t0 + seq_len],
                    pT[:, t0 + chunk:t0 + seq_len],
                    pT[:, t0 + chunk - 1:t0 + seq_len - 1],
                )
                nc.sync.dma_start(out=dx4[C:2 * C, bs], in_=dx4[0:C, bs])
                nc.sync.dma_start(out=dx4[2 * C:4 * C, bs], in_=dx4[0:2 * C, bs])

            def phase3(b, injects=None):
                # outer-product build + contraction with W1 for batch b.
                # W1 contraction interleaved into the E-select stream (chunk 0
                # at lag 2, chunk 1 at lag 4) so PSUM mb tiles drain in time
                # and the chunk-0 accumulator finishes early for the scan.
                # injects[r] emits other phases' work (next praw, the pending
                # store, the batch-after-next pointwise MLP) into the engine
                # queues mid-loop, where it fills dependency-wait holes.
                t0 = b * seq_len
                bs = slice(t0, t0 + seq_len)
                mTr = mTrs[b % 3]
                ups = []
                for _ci in range(cpb):
                    upt = psu.tile([HID, chunk], F32, tag="ups", name=f"ups{_ci}")
                    ups.append(upt)
                rhss = []

                def contract(r, i):
                    nc.tensor.matmul(
                        ups[i][:],
                        cw_sb[:, CW_W1M + r * HID:CW_W1M + (r + 1) * HID],
                        rhss[r][:, i * chunk:(i + 1) * chunk],
                        start=(r == 0), stop=False,
                    )

                for r in range(KT):
                    if injects and r in injects:
                        injects[r]()
                    mb = psm.tile([128, seq_len], F32, tag="mb")
                    for i in range(cpb):
                        nc.tensor.matmul(
                            mb[:, i * chunk:(i + 1) * chunk],
                            cw_sb[:, CW_E + r * 128:CW_E + (r + 1) * 128],
                            mTr[:, i * chunk:(i + 1) * chunk],
                            start=True, stop=True,
                        )
                    rhsb = rpool.tile([128, seq_len], BF16, tag="rhsb")
                    if r in ACT_ROUTE:
                        mbs = mpool.tile([128, seq_len], BF16, tag="mbs")
                        nc.scalar.copy(mbs[:], mb[:])
                        nc.vector.tensor_mul(rhsb[:], mbs[:], dx4[:, bs])
                    elif r in GPS_ROUTE:
                        nc.gpsimd.tensor_mul(rhsb[:], mb[:], dx4[:, bs])
                    else:
                        nc.vector.tensor_mul(rhsb[:], mb[:], dx4[:, bs])
                    rhss.append(rhsb)
                    if r >= 2:
                        contract(r - 2, 0)
                        contract(r - 2, 1)
                # tails: finish chunk 0 first so its scan starts early
                cs0 = slice(t0, t0 + chunk)
                cs1 = slice(t0 + chunk, t0 + seq_len)
                for r in (KT - 2, KT - 1):
                    contract(r, 0)
                    contract(r, 1)
                nc.tensor.matmul(ups[0][:], cw_sb[:, CW_W1DX:CW_W1DX + HID],
                                 dx4[:, cs0], start=False, stop=True)
                nc.tensor.matmul(ups[1][:], cw_sb[:, CW_W1DX:CW_W1DX + HID],
                                 dx4[:, cs1], start=False, stop=True)
                return ups

            def phase4(b, ups, sub):
                # scan straight out of the contraction PSUM accumulators
                # (fp32 carry), then relu+bias, at `sub`-column granularity.
                zb = zpool.tile([HID, seq_len], BF16, tag="zb")
                for si in range(seq_len // sub):
                    ci = (si * sub) // chunk
                    po = si * sub - ci * chunk
                    lo = slice(si * sub, (si + 1) * sub)
                    init = 0.0 if si == 0 else zb[:, si * sub - 1:si * sub]
                    nc.vector.tensor_tensor_scan(
                        zb[:, lo], ups[ci][:, po:po + sub], zsc[:, 0:sub], init,
                        op0=ADD, op1=BYP,
                    )
                    aT = aTs[(2 * b + ci) % 4]
                    nc.scalar.activation(aT[0:HID, po:po + sub], zb[:, lo], RELU,
                                         bias=bb1_sb, scale=1.0)

            def phase5(b, sub):
                # head matmul + bias + store (bb2 added on the PSUM->SBUF
                # copy, so no ones-row in the rhs).  The final batch's copies
                # go on DVE -- idle after the scans -- so the drain isn't
                # paced by the scalar relu4+copy chain.
                t0 = b * seq_len
                last = b == n_batch - 1
                for si in range(seq_len // sub):
                    ci = (si * sub) // chunk
                    po = si * sub - ci * chunk
                    cs = slice(t0 + si * sub, t0 + (si + 1) * sub)
                    aT = aTs[(2 * b + ci) % 4]
                    ops = ps1.tile([DOUT, chunk], F32, tag="ps1t")
                    nc.tensor.matmul(ops[:, 0:sub], cw_sb[:, CW_W2B:CW_W2B + DOUT],
                                     aT[:, po:po + sub], start=True, stop=True)
                    osb = opool.tile([DOUT, sub], BF16, tag="osbt")
                    if last:
                        nc.vector.tensor_scalar_add(osb[:], ops[:, 0:sub],
                                                    bb2_sb)
                    else:
                        nc.scalar.add(osb[:], ops[:, 0:sub], bb2_sb)
                    nc.sync.dma_start(out=out[:, cs], in_=osb[:])

            # ---- software-pipelined schedule ------------------------------
            # All pointwise MLPs run up front (batch-pipelined, so relus
            # overlap the next batch's matmuls); the dx chains drain early so
            # no SBUF-SBUF DMA runs mid-pipeline.  Iterations are then pure
            # phase3/4/5, with the next batch's praw and the pending store
            # injected into the phase3 stream where they fill
            # dependency-wait holes.
            def mtr_pad(i):
                for q in range(3):
                    nc.gpsimd.memset(
                        mTrs[i][C + 32 * q:C + 32 * (q + 1), :], 0.0)

            def at_pad(i):
                for q in range(2):
                    nc.gpsimd.memset(
                        aTs[i][HID + 32 * q:HID + 32 * (q + 1), :], 0.0)

            # gpsimd pad/dx work is emitted in deadline order so its FIFO
            # never delays a praw: mTr1 before batch 1's E-select, aT0/1
            # before phase5(0); the rest is injected into phase3(0) after
            # the first praw.
            for b in range(n_batch):
                phase1(b)
                phase2_dx(b)
                if b == 0:
                    phase2_praw(0, eng=nc.vector)
                    mtr_pad(1)
                if b == 1:
                    at_pad(0)
                    at_pad(1)

            def late_pads():
                mtr_pad(2)
                at_pad(2)
                at_pad(3)

            for it in range(1, n_batch + 1):
                b = it - 1
                injects = {}
                if it < n_batch:
                    # legal from r==0 on: the previous reader of this praw's
                    # mTr tile is phase3(it-2), already emitted
                    injects[0] = (lambda b2=it: phase2_praw(b2))
                if it == 1:
                    injects[3] = late_pads
                if it >= 2:
                    injects[1] = (lambda b5=it - 2: phase5(b5, chunk))
                ups = phase3(b, injects=injects)
                phase4(b, ups, chunk if b < n_batch - 1 else chunk // 2)
                if it == n_batch:
                    phase5(n_batch - 1, chunk // 2)

    nc.compile()
    return nc


def host_prep_shared(w1, b1, w2, b2, W1, bb1, W2, bb2):
    import ml_dtypes
    bf = ml_dtypes.bfloat16
    f = np.float32

    cwm = np.zeros((128, CW_COLS), f)
    cwm[:, CW_W1:CW_W1 + 2 * HID] = (
        np.asarray(w1, f).reshape(2, 128, HID).transpose(1, 0, 2).reshape(128, -1))
    cwm[0:HID, CW_W2:CW_W2 + C] = np.asarray(w2, f)
    # 0.5 midpoint factor folded into W1_sig
    cwm[:, CW_W1M:CW_W1M + KT * HID] = (
        0.5 * np.asarray(W1[C:], f).reshape(KT, 128, HID)
        .transpose(1, 0, 2).reshape(128, -1))
    cwm[0:C, CW_W1DX:CW_W1DX + HID] = np.asarray(W1[:C], f)
    for r in range(KT):
        for q in range(128):
            cwm[4 * r + q // 32, CW_E + 128 * r + q] = 1.0
    cwm[0:HID, CW_W2B:CW_W2B + DOUT] = np.asarray(W2, f)

    cbm = np.zeros((128, 4), f)
    cbm[0:HID, 0] = np.asarray(b1, f)
    cbm[0:C, 1] = np.asarray(b2, f)
    cbm[0:HID, 2] = np.asarray(bb1, f)
    cbm[:, 3] = np.asarray(bb2, f)

    return {
        "cw": cwm.astype(bf),
        "cb": cbm,
    }


_NC_CACHE = {}


def _get_nc():
    key = "full"
    if key not in _NC_CACHE:
        _NC_CACHE[key] = build_nc()
    return _NC_CACHE[key]


def kernel(x, w1, b1, w2, b2, W1, bb1, W2, bb2):
    global LAST_EXEC_NS, LAST_PROFILE, LAST_TRACE_PATH
    import ml_dtypes
    bf = ml_dtypes.bfloat16
    nc = _get_nc()
    shared = host_prep_shared(w1, b1, w2, b2, W1, bb1, W2, bb2)
    xbf = np.ascontiguousarray(x, np.float32).astype(bf)
    n_batch = T // L
    in_maps = []
    for core in range(N_CORES):
        xc = xbf[core * B_CORE:(core + 1) * B_CORE].reshape(T, DIN)
        # (256, T) -> (n_batch, 128, 2, L): [b][p][k][t] = xT[k*128+p, b*L+t]
        xT = xc.T.reshape(2, 128, n_batch, L)
        xTb = np.ascontiguousarray(xT.transpose(2, 1, 0, 3))
        m = dict(shared)
        m["xTb"] = np.ascontiguousarray(xTb)
        in_maps.append(m)
    # rare transient device flakes can surface as NaN output; retry those
    for attempt in range(3):
        try:
            res = bass_utils.run_bass_kernel_spmd(
                nc, in_maps, core_ids=list(range(N_CORES)), trace=TRACE,
            )
        except Exception:
            if not TRACE:
                raise
            res = bass_utils.run_bass_kernel_spmd(
                nc, in_maps, core_ids=list(range(N_CORES)), trace=False,
            )
        LAST_EXEC_NS = res.exec_time_ns
        LAST_PROFILE = res.profile_json
        LAST_TRACE_PATH = (res.instructions_and_trace or (None, None))[1]
        outs = [np.ascontiguousarray(
                    np.asarray(res.results[i]["out"], np.float32).T)
                .reshape(B_CORE, L, DOUT) for i in range(N_CORES)]
        full = np.concatenate(outs, axis=0)
        if np.isfinite(full).all():
            return full
    return full


# revision 47
# speedup vs baseline: 1.1585x; 1.1585x over previous
# Trainium2 Bass kernel for DST_Decoder.
#
# Math reformulation (exact):
#   h  = relu(x @ w1 + b1);  p = h @ w2 + b2                  (pointwise MLP)
#   dx_t = p_t - p_{t-1} (p_{-1}=0);  praw_t = p_t + p_{t-1} = 2*m_t
#   S1_t = p_t;  S2_t = sum_{s<=t} m_s (x) dx_s               (Chen identity)
#   z_t  = cumsum_t[ vec(praw (x) dx) @ (0.5*W1_sig) + dx @ W1_s1 ] + bb1
#   out  = relu(z) @ W2 + bb2
# i.e. contract each timestep's rank-1 outer-product update with W1 FIRST,
# then a cheap 64-wide cumulative scan.  The 0.5 midpoint factor is folded
# into W1_sig on the host so praw needs only an add.
#
# Layout: features on SBUF partitions, time on the free axis; x pre-transposed
# bf16 from the host.  The outer-product tensor O^T (1024, t) is built k-tile
# by k-tile: PE broadcasts rows of praw^T to 128 partitions via a 0/1
# selection matrix (E_r @ praw^T -> 2-bank PSUM tile), then the product with
# a 4x-stacked bf16 copy of dx^T is formed on DVE / gpsimd / (Scalar copy +
# DVE 2x-bf16) split across k-tiles for engine balance.  All matmul operands
# are bf16 and every matmul is padded to contraction dim K=128; pad rows hold
# zero weights and rhs pad rows are zeroed ON-CHIP once per rotating buffer
# (memset is column-bound, so pads live in narrow per-batch/per-chunk tiles).
# The 64-wide scan reads the contraction PSUM accumulator directly.  All x
# batches stream in up front and every pointwise MLP runs before the first
# outer-product block, so mid-pipeline DMA is just the dx 4x-stacking and the
# (bf16) output stores.  The W1 contraction is interleaved into the E-select
# stream (lag 2) so PSUM mb tiles drain in time; the next batch's praw and
# the pending store are injected into the phase3 stream where they fill
# dependency-wait holes; the last batch drains at 256-column granularity.
# Sharding: data-parallel over batch, 4 batches per core, weights replicated.

import os
import sys

import numpy as np

for _p in ("/opt/trn_rl_repo",):
    if _p not in sys.path and os.path.isdir(_p):
        sys.path.append(_p)

from concourse import bacc, tile
from concourse import bass_utils
import concourse.mybir as mybir

F32 = mybir.dt.float32
BF16 = mybir.dt.bfloat16

N_CORES = 8
B, L, DIN = 32, 1024, 256
C, HID, DOUT = 32, 64, 128
B_CORE = B // N_CORES                 # 4 batches per core
T = B_CORE * L                        # 4096 time positions per core
KT = (C * C) // 128                   # 8 k-tiles of the outer-product block
ACT_ROUTE = (3, 5, 7)                 # k-tiles multiplied on DVE after ACT copy
GPS_ROUTE = ()                        # gpsimd cannot read PSUM mb tiles

# packed const layout (columns in cw)
CW_W1 = 0                             # 2*HID
CW_W2 = 128                           # C
CW_W1M = 160                          # KT*HID
CW_W1DX = 672                         # HID
CW_E = 736                            # KT*128
CW_W2B = 1760                         # DOUT
CW_COLS = 1888
CW_P1 = 160                           # phase1 needs only the first 160 cols

TRACE = False
LAST_EXEC_NS = None
LAST_PROFILE = None
LAST_TRACE_PATH = None


def build_nc(t_total=T, seq_len=L, chunk=512):
    n_batch = t_total // seq_len      # 4 batches
    cpb = seq_len // chunk            # 2 chunks per batch

    nc = bacc.Bacc(trn_type="TRN2", target_bir_lowering=False, debug=False)

    xTb = nc.dram_tensor("xTb", (n_batch, 128, 2, seq_len), BF16,
                         kind="ExternalInput").ap()
    cw = nc.dram_tensor("cw", (128, CW_COLS), BF16, kind="ExternalInput").ap()
    cb = nc.dram_tensor("cb", (128, 4), F32, kind="ExternalInput").ap()
    out = nc.dram_tensor("out", (DOUT, t_total), BF16,
                         kind="ExternalOutput").ap()

    RELU = mybir.ActivationFunctionType.Relu
    ADD = mybir.AluOpType.add
    BYP = mybir.AluOpType.bypass

    with tile.TileContext(nc) as tc:
        with (
            tc.tile_pool(name="consts", bufs=1) as cpool,
            tc.tile_pool(name="persist", bufs=1) as ppool,
            tc.tile_pool(name="xin", bufs=4) as xpool,
            tc.tile_pool(name="mbs", bufs=2) as mpool,
            tc.tile_pool(name="rhs", bufs=8) as rpool,
            tc.tile_pool(name="zbuf", bufs=2) as zpool,
            tc.tile_pool(name="obuf", bufs=6) as opool,
            tc.tile_pool(name="ps1", bufs=2, space="PSUM") as ps1,
            tc.tile_pool(name="psu", bufs=2, space="PSUM") as psu,
            tc.tile_pool(name="psm", bufs=2, space="PSUM") as psm,
        ):
            # ---- input DMAs: everything streams in up front; batch-0 x is
            # split 4 ways so the very first matmul gates on 128KB ----------
            xts = {}

            def issue_x(b, split=False):
                xt = xpool.tile([128, 2, seq_len], BF16, tag="xt")
                if split:
                    nc.sync.dma_start(out=xt[:, 0, 0:chunk],
                                      in_=xTb[b][:, 0, 0:chunk])
                    return xt
                nc.sync.dma_start(out=xt[:], in_=xTb[b])
                xts[b] = xt

            xt0 = issue_x(0, split=True)
            cw_sb = cpool.tile([128, CW_COLS], BF16, tag="cw")
            nc.sync.dma_start(out=cw_sb[:, 0:CW_P1], in_=cw[:, 0:CW_P1])
            nc.sync.dma_start(out=xt0[:, 1, 0:chunk], in_=xTb[0][:, 1, 0:chunk])
            nc.sync.dma_start(out=xt0[:, 0, chunk:seq_len],
                              in_=xTb[0][:, 0, chunk:seq_len])
            nc.sync.dma_start(out=xt0[:, 1, chunk:seq_len],
                              in_=xTb[0][:, 1, chunk:seq_len])
            xts[0] = xt0
            cb_sb = cpool.tile([128, 4], F32, tag="cb")
            nc.sync.dma_start(out=cb_sb[:], in_=cb)
            nc.sync.dma_start(out=cw_sb[:, CW_P1:CW_COLS],
                              in_=cw[:, CW_P1:CW_COLS])
            for b in range(1, n_batch):
                issue_x(b)
            b1_sb = cb_sb[0:HID, 0:1]
            b2_sb = cb_sb[0:C, 1:2]
            bb1_sb = cb_sb[0:HID, 2:3]
            bb2_sb = cb_sb[:, 3:4]

            # ---- persistent activations -----------------------------------
            pT = ppool.tile([C, t_total], F32, tag="pT")
            dx4 = ppool.tile([128, t_total], BF16, tag="dx4")
            hsbP = ppool.tile([128, 2, chunk], BF16, tag="hsbP")
            zsc = ppool.tile([HID, chunk], BF16, tag="zsc")
            # praw tiles rotate per batch, aT tiles per chunk; their pad rows
            # (zero weights in the matmuls, just need finite values) are
            # zeroed once per buffer here instead of per batch.
            mTrs = [ppool.tile([128, seq_len], BF16, tag=f"mTr{i}",
                               name=f"mTr{i}") for i in range(3)]
            aTs = [ppool.tile([128, chunk], BF16, tag=f"aTc{i}",
                              name=f"aT{i}") for i in range(4)]

            # small pads on DVE (fast, nothing queued behind them yet)
            for q in range(2):
                nc.vector.memset(hsbP[HID + 32 * q:HID + 32 * (q + 1), 0, :], 0.0)
                nc.vector.memset(hsbP[HID + 32 * q:HID + 32 * (q + 1), 1, :], 0.0)
            nc.vector.memset(zsc[:], 0.0)
            # batch-0 praw pads on gpsimd ahead of batch-0's dx diff
            for q in range(3):
                nc.gpsimd.memset(mTrs[0][C + 32 * q:C + 32 * (q + 1), :], 0.0)

            def phase1(b):
                # pointwise MLP for both chunks of batch b (x prefetched).
                # relu+bias on DVE: phase1 runs only in the up-front window
                # where DVE is idle, and the scalar queue (ACT table load +
                # bias adds) was pacing the whole chain.
                xt = xts.pop(b)
                for ci in range(cpb):
                    hps = ps1.tile([HID, chunk], F32, tag="ps1t")
                    for k in range(2):
                        nc.tensor.matmul(hps[:],
                                         cw_sb[:, CW_W1 + k * HID:
                                               CW_W1 + (k + 1) * HID],
                                         xt[:, k, ci * chunk:(ci + 1) * chunk],
                                         start=(k == 0), stop=(k == 1))
                    nc.scalar.activation(hsbP[0:HID, ci, :], hps[:], RELU,
                                         bias=b1_sb, scale=1.0)
                for ci in range(cpb):
                    cs = slice(b * seq_len + ci * chunk,
                               b * seq_len + (ci + 1) * chunk)
                    pps = ps1.tile([C, chunk], F32, tag="ps1t")
                    nc.tensor.matmul(pps[:], cw_sb[:, CW_W2:CW_W2 + C],
                                     hsbP[:, ci, :], start=True, stop=True)
                    nc.scalar.add(pT[:, cs], pps[:], b2_sb)

            def phase2_praw(b, eng=None):
                # praw (feeds the E-select broadcast); during iterations
                # gpsimd is otherwise idle, so it runs there by default --
                # only batch 0's (on the critical path to the first E-select)
                # goes on DVE.
                eng = eng or nc.gpsimd
                t0 = b * seq_len
                mTr = mTrs[b % 3]
                eng.tensor_copy(mTr[0:C, 0:1], pT[:, t0:t0 + 1])
                eng.tensor_add(
                    mTr[0:C, 1:seq_len],
                    pT[:, t0 + 1:t0 + seq_len],
                    pT[:, t0:t0 + seq_len - 1],
                )

            def phase2_dx(b):
                # dx diff straight to bf16 dx4 rows 0:32 on gpsimd (split per
                # chunk), then 4x partition-stacking via two log-doubling
                # SBUF-to-SBUF DMAs
                t0 = b * seq_len
                bs = slice(t0, t0 + seq_len)
                nc.gpsimd.tensor_copy(dx4[0:C, t0:t0 + 1], pT[:, t0:t0 + 1])
                nc.gpsimd.tensor_sub(
                    dx4[0:C, t0 + 1:t0 + chunk],
                    pT[:, t0 + 1:t0 + chunk],
                    pT[:, t0:t0 + chunk - 1],
                )
                nc.gpsimd.tensor_sub(
                    dx4[0:C, t0 + chunk:t0 + seq_len],
                    pT[:, t0 + chunk:t0 + seq_len],
                    pT[:, t0 + chunk - 1:t0 + seq_len - 1],
                )
                nc.sync.dma_start(out=dx4[C:2 * C, bs], in_=dx4[0:C, bs])
                nc.sync.dma_start(out=dx4[2 * C:4 * C, bs], in_=dx4[0:2 * C, bs])

            def phase3(b, injects=None):
                # outer-product build + contraction with W1 for batch b.
                # W1 contraction interleaved into the E-select stream (chunk 0
                # at lag 2, chunk 1 at lag 4) so PSUM mb tiles drain in time
                # and the chunk-0 accumulator finishes early for the scan.
                # injects[r] emits other phases' work (next praw, the pending
                # store, the batch-after-next pointwise MLP) into the engine
                # queues mid-loop, where it fills dependency-wait holes.
                t0 = b * seq_len
                bs = slice(t0, t0 + seq_len)
                mTr = mTrs[b % 3]
                ups = []
                for _ci in range(cpb):
                    upt = psu.tile([HID, chunk], F32, tag="ups", name=f"ups{_ci}")
                    ups.append(upt)
                rhss = []

                def contract(r, i):
                    nc.tensor.matmul(
                        ups[i][:],
                        cw_sb[:, CW_W1M + r * HID:CW_W1M + (r + 1) * HID],
                        rhss[r][:, i * chunk:(i + 1) * chunk],
                        start=(r == 0), stop=False,
                    )

                for r in range(KT):
                    if injects and r in injects:
                        injects[r]()
                    mb = psm.tile([128, seq_len], F32, tag="mb")
                    for i in range(cpb):
                        nc.tensor.matmul(
                            mb[:, i * chunk:(i + 1) * chunk],
                            cw_sb[:, CW_E + r * 128:CW_E + (r + 1) * 128],
                            mTr[:, i * chunk:(i + 1) * chunk],
                            start=True, stop=True,
                        )
                    rhsb = rpool.tile([128, seq_len], BF16, tag="rhsb")
                    if r in ACT_ROUTE:
                        mbs = mpool.tile([128, seq_len], BF16, tag="mbs")
                        nc.scalar.copy(mbs[:], mb[:])
                        nc.vector.tensor_mul(rhsb[:], mbs[:], dx4[:, bs])
                    elif r in GPS_ROUTE:
                        nc.gpsimd.tensor_mul(rhsb[:], mb[:], dx4[:, bs])
                    else:
                        nc.vector.tensor_mul(rhsb[:], mb[:], dx4[:, bs])
                    rhss.append(rhsb)
                    if r >= 2:
                        contract(r - 2, 0)
                        contract(r - 2, 1)
                # tails: finish chunk 0 first so its scan starts early
                cs0 = slice(t0, t0 + chunk)
                cs1 = slice(t0 + chunk, t0 + seq_len)
                for r in (KT - 2, KT - 1):
                    contract(r, 0)
                    contract(r, 1)
                nc.tensor.matmul(ups[0][:], cw_sb[:, CW_W1DX:CW_W1DX + HID],
                                 dx4[:, cs0], start=False, stop=True)
                nc.tensor.matmul(ups[1][:], cw_sb[:, CW_W1DX:CW_W1DX + HID],
                                 dx4[:, cs1], start=False, stop=True)
                return ups

            def phase4(b, ups, sub):
                # scan straight out of the contraction PSUM accumulators
                # (fp32 carry), then relu+bias, at `sub`-column granularity.
                zb = zpool.tile([HID, seq_len], BF16, tag="zb")
                for si in range(seq_len // sub):
                    ci = (si * sub) // chunk
                    po = si * sub - ci * chunk
                    lo = slice(si * sub, (si + 1) * sub)
                    init = 0.0 if si == 0 else zb[:, si * sub - 1:si * sub]
                    nc.vector.tensor_tensor_scan(
                        zb[:, lo], ups[ci][:, po:po + sub], zsc[:, 0:sub], init,
                        op0=ADD, op1=BYP,
                    )
                    aT = aTs[(2 * b + ci) % 4]
                    nc.scalar.activation(aT[0:HID, po:po + sub], zb[:, lo], RELU,
                                         bias=bb1_sb, scale=1.0)

            def phase5(b, sub):
                # head matmul + bias + store (bb2 added on the PSUM->SBUF
                # copy, so no ones-row in the rhs).  The final batch's copies
                # go on DVE -- idle after the scans -- so the drain isn't
                # paced by the scalar relu4+copy chain.
                t0 = b * seq_len
                last = b == n_batch - 1
                for si in range(seq_len // sub):
                    ci = (si * sub) // chunk
                    po = si * sub - ci * chunk
                    cs = slice(t0 + si * sub, t0 + (si + 1) * sub)
                    aT = aTs[(2 * b + ci) % 4]
                    ops = ps1.tile([DOUT, chunk], F32, tag="ps1t")
                    nc.tensor.matmul(ops[:, 0:sub], cw_sb[:, CW_W2B:CW_W2B + DOUT],
                                     aT[:, po:po + sub], start=True, stop=True)
                    osb = opool.tile([DOUT, sub], BF16, tag="osbt")
                    if last:
                        nc.vector.tensor_scalar_add(osb[:], ops[:, 0:sub],
                                                    bb2_sb)
                    else:
                        nc.scalar.add(osb[:], ops[:, 0:sub], bb2_sb)
                    nc.sync.dma_start(out=out[:, cs], in_=osb[:])

            # ---- software-pipelined schedule ------------------------------
            # All pointwise MLPs run up front (batch-pipelined, so relus
            # overlap the next batch's matmuls); the dx chains drain early so
            # no SBUF-SBUF DMA runs mid-pipeline.  Iterations are then pure
            # phase3/4/5, with the next batch's praw and the pending store
            # injected into the phase3 stream where they fill
            # dependency-wait holes.
            def mtr_pad(i):
                for q in range(3):
                    nc.gpsimd.memset(
                        mTrs[i][C + 32 * q:C + 32 * (q + 1), :], 0.0)

            def at_pad(i):
                for q in range(2):
                    nc.gpsimd.memset(
                        aTs[i][HID + 32 * q:HID + 32 * (q + 1), :], 0.0)

            # gpsimd pad/dx work is emitted in deadline order so its FIFO
            # never delays a praw: mTr1 before batch 1's E-select, aT0/1
            # before phase5(0); the rest is injected into phase3(0) after
            # the first praw.
            for b in range(n_batch):
                phase1(b)
                phase2_dx(b)
                if b == 0:
                    phase2_praw(0, eng=nc.vector)
                    mtr_pad(1)
                if b == 1:
                    at_pad(0)
                    at_pad(1)

            def late_pads():
                mtr_pad(2)
                at_pad(2)
                at_pad(3)

            for it in range(1, n_batch + 1):
                b = it - 1
                injects = {}
                if it < n_batch:
                    # legal from r==0 on: the previous reader of this praw's
                    # mTr tile is phase3(it-2), already emitted
                    injects[0] = (lambda b2=it: phase2_praw(b2))
                if it == 1:
                    injects[3] = late_pads
                if it >= 2:
                    injects[1] = (lambda b5=it - 2: phase5(b5, chunk))
                ups = phase3(b, injects=injects)
                phase4(b, ups, chunk if b < n_batch - 1 else chunk // 2)
                if it == n_batch:
                    phase5(n_batch - 1, chunk // 2)

    nc.compile()
    return nc


def host_prep_shared(w1, b1, w2, b2, W1, bb1, W2, bb2):
    import ml_dtypes
    bf = ml_dtypes.bfloat16
    f = np.float32

    cwm = np.zeros((128, CW_COLS), f)
    cwm[:, CW_W1:CW_W1 + 2 * HID] = (
        np.asarray(w1, f).reshape(2, 128, HID).transpose(1, 0, 2).reshape(128, -1))
    cwm[0:HID, CW_W2:CW_W2 + C] = np.asarray(w2, f)
    # 0.5 midpoint factor folded into W1_sig
    cwm[:, CW_W1M:CW_W1M + KT * HID] = (
        0.5 * np.asarray(W1[C:], f).reshape(KT, 128, HID)
        .transpose(1, 0, 2).reshape(128, -1))
    cwm[0:C, CW_W1DX:CW_W1DX + HID] = np.asarray(W1[:C], f)
    for r in range(KT):
        for q in range(128):
            cwm[4 * r + q // 32, CW_E + 128 * r + q] = 1.0
    cwm[0:HID, CW_W2B:CW_W2B + DOUT] = np.asarray(W2, f)

    cbm = np.zeros((128, 4), f)
    cbm[0:HID, 0] = np.asarray(b1, f)
    cbm[0:C, 1] = np.asarray(b2, f)
    cbm[0:HID, 2] = np.asarray(bb1, f)
    cbm[:, 3] = np.asarray(bb2, f)

    return {
        "cw": cwm.astype(bf),
        "cb": cbm,
    }


_NC_CACHE = {}


def _get_nc():
    key = "full"
    if key not in _NC_CACHE:
        _NC_CACHE[key] = build_nc()
    return _NC_CACHE[key]


def kernel(x, w1, b1, w2, b2, W1, bb1, W2, bb2):
    global LAST_EXEC_NS, LAST_PROFILE, LAST_TRACE_PATH
    import ml_dtypes
    bf = ml_dtypes.bfloat16
    nc = _get_nc()
    shared = host_prep_shared(w1, b1, w2, b2, W1, bb1, W2, bb2)
    xbf = np.ascontiguousarray(x, np.float32).astype(bf)
    n_batch = T // L
    in_maps = []
    for core in range(N_CORES):
        xc = xbf[core * B_CORE:(core + 1) * B_CORE].reshape(T, DIN)
        # (256, T) -> (n_batch, 128, 2, L): [b][p][k][t] = xT[k*128+p, b*L+t]
        xT = xc.T.reshape(2, 128, n_batch, L)
        xTb = np.ascontiguousarray(xT.transpose(2, 1, 0, 3))
        m = dict(shared)
        m["xTb"] = np.ascontiguousarray(xTb)
        in_maps.append(m)
    # rare transient device flakes can surface as NaN output; retry those
    for attempt in range(3):
        try:
            res = bass_utils.run_bass_kernel_spmd(
                nc, in_maps, core_ids=list(range(N_CORES)), trace=TRACE,
            )
        except Exception:
            if not TRACE:
                raise
            res = bass_utils.run_bass_kernel_spmd(
                nc, in_maps, core_ids=list(range(N_CORES)), trace=False,
            )
        LAST_EXEC_NS = res.exec_time_ns
        LAST_PROFILE = res.profile_json
        LAST_TRACE_PATH = (res.instructions_and_trace or (None, None))[1]
        outs = [np.ascontiguousarray(
                    np.asarray(res.results[i]["out"], np.float32).T)
                .reshape(B_CORE, L, DOUT) for i in range(N_CORES)]
        full = np.concatenate(outs, axis=0)
        if np.isfinite(full).all():
            return full
    return full


# revision 54
# speedup vs baseline: 1.1652x; 1.0058x over previous
# Trainium2 Bass kernel for DST_Decoder.
#
# Math reformulation (exact):
#   h  = relu(x @ w1 + b1);  p = h @ w2 + b2                  (pointwise MLP)
#   dx_t = p_t - p_{t-1} (p_{-1}=0);  praw_t = p_t + p_{t-1} = 2*m_t
#   S1_t = p_t;  S2_t = sum_{s<=t} m_s (x) dx_s               (Chen identity)
#   z_t  = cumsum_t[ vec(praw (x) dx) @ (0.5*W1_sig) + dx @ W1_s1 ] + bb1
#   out  = relu(z) @ W2 + bb2
# i.e. contract each timestep's rank-1 outer-product update with W1 FIRST,
# then a cheap 64-wide cumulative scan.  The 0.5 midpoint factor is folded
# into W1_sig on the host so praw needs only an add.
#
# Layout: features on SBUF partitions, time on the free axis; x pre-transposed
# bf16 from the host.  The outer-product tensor O^T (1024, t) is built k-tile
# by k-tile: PE broadcasts rows of praw^T to 128 partitions via a 0/1
# selection matrix (E_r @ praw^T -> 2-bank PSUM tile), then the product with
# a 4x-stacked bf16 copy of dx^T is formed on DVE / gpsimd / (Scalar copy +
# DVE 2x-bf16) split across k-tiles for engine balance.  All matmul operands
# are bf16 and every matmul is padded to contraction dim K=128; pad rows hold
# zero weights and rhs pad rows are zeroed ON-CHIP once per rotating buffer
# (memset is column-bound, so pads live in narrow per-batch/per-chunk tiles).
# The 64-wide scan reads the contraction PSUM accumulator directly.  All x
# batches stream in up front and every pointwise MLP runs before the first
# outer-product block, so mid-pipeline DMA is just the dx 4x-stacking and the
# (bf16) output stores.  The W1 contraction is interleaved into the E-select
# stream (lag 2) so PSUM mb tiles drain in time; the next batch's praw and
# the pending store are injected into the phase3 stream where they fill
# dependency-wait holes; the last batch drains at 256-column granularity.
# Sharding: data-parallel over batch, 4 batches per core, weights replicated.

import os
import sys

import numpy as np

for _p in ("/opt/trn_rl_repo",):
    if _p not in sys.path and os.path.isdir(_p):
        sys.path.append(_p)

from concourse import bacc, tile
from concourse import bass_utils
import concourse.mybir as mybir

F32 = mybir.dt.float32
BF16 = mybir.dt.bfloat16

N_CORES = 8
B, L, DIN = 32, 1024, 256
C, HID, DOUT = 32, 64, 128
B_CORE = B // N_CORES                 # 4 batches per core
T = B_CORE * L                        # 4096 time positions per core
KT = (C * C) // 128                   # 8 k-tiles of the outer-product block
ACT_ROUTE = (3, 5, 7)                 # k-tiles multiplied from an ACT bf16 copy
GPS_MUL = ()                          # gpsimd bf16*bf16 multiply miscomputes;
                                      # keep the routed multiplies on DVE

# packed const layout (columns in cw)
CW_W1 = 0                             # 2*HID
CW_W2 = 128                           # C
CW_W1M = 160                          # KT*HID
CW_W1DX = 672                         # HID
CW_E = 736                            # KT*128
CW_W2B = 1760                         # DOUT
CW_COLS = 1888
CW_P1 = 160                           # phase1 needs only the first 160 cols

TRACE = False
LAST_EXEC_NS = None
LAST_PROFILE = None
LAST_TRACE_PATH = None


def build_nc(t_total=T, seq_len=L, chunk=512):
    n_batch = t_total // seq_len      # 4 batches
    cpb = seq_len // chunk            # 2 chunks per batch

    nc = bacc.Bacc(trn_type="TRN2", target_bir_lowering=False, debug=False)

    xTb = nc.dram_tensor("xTb", (n_batch, 128, 2, seq_len), BF16,
                         kind="ExternalInput").ap()
    cw = nc.dram_tensor("cw", (128, CW_COLS), BF16, kind="ExternalInput").ap()
    cb = nc.dram_tensor("cb", (128, 4), F32, kind="ExternalInput").ap()
    out = nc.dram_tensor("out", (DOUT, t_total), BF16,
                         kind="ExternalOutput").ap()

    RELU = mybir.ActivationFunctionType.Relu
    ADD = mybir.AluOpType.add
    BYP = mybir.AluOpType.bypass

    with tile.TileContext(nc) as tc:
        with (
            tc.tile_pool(name="consts", bufs=1) as cpool,
            tc.tile_pool(name="persist", bufs=1) as ppool,
            tc.tile_pool(name="xin", bufs=4) as xpool,
            tc.tile_pool(name="mbs", bufs=2) as mpool,
            tc.tile_pool(name="rhs", bufs=8) as rpool,
            tc.tile_pool(name="zbuf", bufs=2) as zpool,
            tc.tile_pool(name="obuf", bufs=6) as opool,
            tc.tile_pool(name="ps1", bufs=2, space="PSUM") as ps1,
            tc.tile_pool(name="psu", bufs=2, space="PSUM") as psu,
            tc.tile_pool(name="psm", bufs=2, space="PSUM") as psm,
        ):
            # ---- input DMAs: everything streams in up front; batch-0 x is
            # split 4 ways so the very first matmul gates on 128KB ----------
            xts = {}

            def issue_x(b, split=False):
                xt = xpool.tile([128, 2, seq_len], BF16, tag="xt")
                if split:
                    nc.sync.dma_start(out=xt[:, 0, 0:chunk],
                                      in_=xTb[b][:, 0, 0:chunk])
                    return xt
                nc.sync.dma_start(out=xt[:], in_=xTb[b])
                xts[b] = xt

            xt0 = issue_x(0, split=True)
            cw_sb = cpool.tile([128, CW_COLS], BF16, tag="cw")
            nc.sync.dma_start(out=cw_sb[:, 0:CW_P1], in_=cw[:, 0:CW_P1])
            nc.sync.dma_start(out=xt0[:, 1, 0:chunk], in_=xTb[0][:, 1, 0:chunk])
            nc.sync.dma_start(out=xt0[:, 0, chunk:seq_len],
                              in_=xTb[0][:, 0, chunk:seq_len])
            nc.sync.dma_start(out=xt0[:, 1, chunk:seq_len],
                              in_=xTb[0][:, 1, chunk:seq_len])
            xts[0] = xt0
            cb_sb = cpool.tile([128, 4], F32, tag="cb")
            nc.sync.dma_start(out=cb_sb[:], in_=cb)
            nc.sync.dma_start(out=cw_sb[:, CW_P1:CW_COLS],
                              in_=cw[:, CW_P1:CW_COLS])
            for b in range(1, n_batch):
                issue_x(b)
            b1_sb = cb_sb[0:HID, 0:1]
            b2_sb = cb_sb[0:C, 1:2]
            bb1_sb = cb_sb[0:HID, 2:3]
            bb2_sb = cb_sb[:, 3:4]

            # ---- persistent activations -----------------------------------
            pT = ppool.tile([C, t_total], F32, tag="pT")
            dx4 = ppool.tile([128, t_total], BF16, tag="dx4")
            hsbP = ppool.tile([128, 2, chunk], BF16, tag="hsbP")
            zsc = ppool.tile([HID, chunk], BF16, tag="zsc")
            # praw tiles rotate per batch, aT tiles per chunk; their pad rows
            # (zero weights in the matmuls, just need finite values) are
            # zeroed once per buffer here instead of per batch.
            mTrs = [ppool.tile([128, seq_len], BF16, tag=f"mTr{i}",
                               name=f"mTr{i}") for i in range(3)]
            aTs = [ppool.tile([128, chunk], BF16, tag=f"aTc{i}",
                              name=f"aT{i}") for i in range(4)]

            # small pads on DVE (fast, nothing queued behind them yet)
            for q in range(2):
                nc.vector.memset(hsbP[HID + 32 * q:HID + 32 * (q + 1), 0, :], 0.0)
                nc.vector.memset(hsbP[HID + 32 * q:HID + 32 * (q + 1), 1, :], 0.0)
            nc.vector.memset(zsc[:], 0.0)
            # batch-0 praw pads on gpsimd ahead of batch-0's dx diff
            for q in range(3):
                nc.gpsimd.memset(mTrs[0][C + 32 * q:C + 32 * (q + 1), :], 0.0)

            def phase1(b):
                # pointwise MLP for both chunks of batch b (x prefetched).
                # relu+bias on DVE: phase1 runs only in the up-front window
                # where DVE is idle, and the scalar queue (ACT table load +
                # bias adds) was pacing the whole chain.
                xt = xts.pop(b)
                for ci in range(cpb):
                    hps = ps1.tile([HID, chunk], F32, tag="ps1t")
                    for k in range(2):
                        nc.tensor.matmul(hps[:],
                                         cw_sb[:, CW_W1 + k * HID:
                                               CW_W1 + (k + 1) * HID],
                                         xt[:, k, ci * chunk:(ci + 1) * chunk],
                                         start=(k == 0), stop=(k == 1))
                    nc.scalar.activation(hsbP[0:HID, ci, :], hps[:], RELU,
                                         bias=b1_sb, scale=1.0)
                for ci in range(cpb):
                    cs = slice(b * seq_len + ci * chunk,
                               b * seq_len + (ci + 1) * chunk)
                    pps = ps1.tile([C, chunk], F32, tag="ps1t")
                    nc.tensor.matmul(pps[:], cw_sb[:, CW_W2:CW_W2 + C],
                                     hsbP[:, ci, :], start=True, stop=True)
                    nc.scalar.add(pT[:, cs], pps[:], b2_sb)

            def phase2_praw(b, eng=None):
                # praw (feeds the E-select broadcast); during iterations
                # gpsimd is otherwise idle, so it runs there by default --
                # only batch 0's (on the critical path to the first E-select)
                # goes on DVE.
                eng = eng or nc.gpsimd
                t0 = b * seq_len
                mTr = mTrs[b % 3]
                eng.tensor_copy(mTr[0:C, 0:1], pT[:, t0:t0 + 1])
                eng.tensor_add(
                    mTr[0:C, 1:seq_len],
                    pT[:, t0 + 1:t0 + seq_len],
                    pT[:, t0:t0 + seq_len - 1],
                )

            def phase2_dx(b):
                # dx diff straight to bf16 dx4 rows 0:32 on gpsimd (split per
                # chunk), then 4x partition-stacking via two log-doubling
                # SBUF-to-SBUF DMAs
                t0 = b * seq_len
                bs = slice(t0, t0 + seq_len)
                nc.gpsimd.tensor_copy(dx4[0:C, t0:t0 + 1], pT[:, t0:t0 + 1])
                nc.gpsimd.tensor_sub(
                    dx4[0:C, t0 + 1:t0 + chunk],
                    pT[:, t0 + 1:t0 + chunk],
                    pT[:, t0:t0 + chunk - 1],
                )
                nc.gpsimd.tensor_sub(
                    dx4[0:C, t0 + chunk:t0 + seq_len],
                    pT[:, t0 + chunk:t0 + seq_len],
                    pT[:, t0 + chunk - 1:t0 + seq_len - 1],
                )
                nc.sync.dma_start(out=dx4[C:2 * C, bs], in_=dx4[0:C, bs])
                nc.sync.dma_start(out=dx4[2 * C:4 * C, bs], in_=dx4[0:2 * C, bs])

            def phase3(b, injects=None):
                # outer-product build + contraction with W1 for batch b.
                # W1 contraction interleaved into the E-select stream (chunk 0
                # at lag 2, chunk 1 at lag 4) so PSUM mb tiles drain in time
                # and the chunk-0 accumulator finishes early for the scan.
                # injects[r] emits other phases' work (next praw, the pending
                # store, the batch-after-next pointwise MLP) into the engine
                # queues mid-loop, where it fills dependency-wait holes.
                t0 = b * seq_len
                bs = slice(t0, t0 + seq_len)
                mTr = mTrs[b % 3]
                ups = []
                for _ci in range(cpb):
                    upt = psu.tile([HID, chunk], F32, tag="ups", name=f"ups{_ci}")
                    ups.append(upt)
                rhss = []

                def contract(r, i):
                    nc.tensor.matmul(
                        ups[i][:],
                        cw_sb[:, CW_W1M + r * HID:CW_W1M + (r + 1) * HID],
                        rhss[r][:, i * chunk:(i + 1) * chunk],
                        start=(r == 0), stop=False,
                    )

                for r in range(KT):
                    if injects and r in injects:
                        injects[r]()
                    mb = psm.tile([128, seq_len], F32, tag="mb")
                    for i in range(cpb):
                        nc.tensor.matmul(
                            mb[:, i * chunk:(i + 1) * chunk],
                            cw_sb[:, CW_E + r * 128:CW_E + (r + 1) * 128],
                            mTr[:, i * chunk:(i + 1) * chunk],
                            start=True, stop=True,
                        )
                    rhsb = rpool.tile([128, seq_len], BF16, tag="rhsb")
                    if r in ACT_ROUTE:
                        mbs = mpool.tile([128, seq_len], BF16, tag="mbs")
                        nc.scalar.copy(mbs[:], mb[:])
                        eng = nc.gpsimd if r in GPS_MUL else nc.vector
                        eng.tensor_mul(rhsb[:], mbs[:], dx4[:, bs])
                    else:
                        nc.vector.tensor_mul(rhsb[:], mb[:], dx4[:, bs])
                    rhss.append(rhsb)
                    if r >= 2:
                        contract(r - 2, 0)
                        contract(r - 2, 1)
                # tails: finish chunk 0 first so its scan starts early
                cs0 = slice(t0, t0 + chunk)
                cs1 = slice(t0 + chunk, t0 + seq_len)
                for r in (KT - 2, KT - 1):
                    contract(r, 0)
                    contract(r, 1)
                nc.tensor.matmul(ups[0][:], cw_sb[:, CW_W1DX:CW_W1DX + HID],
                                 dx4[:, cs0], start=False, stop=True)
                nc.tensor.matmul(ups[1][:], cw_sb[:, CW_W1DX:CW_W1DX + HID],
                                 dx4[:, cs1], start=False, stop=True)
                return ups

            def phase4(b, ups, sub):
                # scan straight out of the contraction PSUM accumulators
                # (fp32 carry), then relu+bias, at `sub`-column granularity.
                zb = zpool.tile([HID, seq_len], BF16, tag="zb")
                for si in range(seq_len // sub):
                    ci = (si * sub) // chunk
                    po = si * sub - ci * chunk
                    lo = slice(si * sub, (si + 1) * sub)
                    init = 0.0 if si == 0 else zb[:, si * sub - 1:si * sub]
                    nc.vector.tensor_tensor_scan(
                        zb[:, lo], ups[ci][:, po:po + sub], zsc[:, 0:sub], init,
                        op0=ADD, op1=BYP,
                    )
                    aT = aTs[(2 * b + ci) % 4]
                    nc.scalar.activation(aT[0:HID, po:po + sub], zb[:, lo], RELU,
                                         bias=bb1_sb, scale=1.0)

            def phase5(b, sub):
                # head matmul + bias + store (bb2 added on the PSUM->SBUF
                # copy, so no ones-row in the rhs).  The final batch's copies
                # go on DVE -- idle after the scans -- so the drain isn't
                # paced by the scalar relu4+copy chain.
                t0 = b * seq_len
                last = b == n_batch - 1
                for si in range(seq_len // sub):
                    ci = (si * sub) // chunk
                    po = si * sub - ci * chunk
                    cs = slice(t0 + si * sub, t0 + (si + 1) * sub)
                    aT = aTs[(2 * b + ci) % 4]
                    ops = ps1.tile([DOUT, chunk], F32, tag="ps1t")
                    nc.tensor.matmul(ops[:, 0:sub], cw_sb[:, CW_W2B:CW_W2B + DOUT],
                                     aT[:, po:po + sub], start=True, stop=True)
                    osb = opool.tile([DOUT, sub], BF16, tag="osbt")
                    if last:
                        nc.vector.tensor_scalar_add(osb[:], ops[:, 0:sub],
                                                    bb2_sb)
                    else:
                        nc.scalar.add(osb[:], ops[:, 0:sub], bb2_sb)
                    nc.sync.dma_start(out=out[:, cs], in_=osb[:])

            # ---- software-pipelined schedule ------------------------------
            # All pointwise MLPs run up front (batch-pipelined, so relus
            # overlap the next batch's matmuls); the dx chains drain early so
            # no SBUF-SBUF DMA runs mid-pipeline.  Iterations are then pure
            # phase3/4/5, with the next batch's praw and the pending store
            # injected into the phase3 stream where they fill
            # dependency-wait holes.
            def mtr_pad(i):
                for q in range(3):
                    nc.gpsimd.memset(
                        mTrs[i][C + 32 * q:C + 32 * (q + 1), :], 0.0)

            def at_pad(i):
                for q in range(2):
                    nc.gpsimd.memset(
                        aTs[i][HID + 32 * q:HID + 32 * (q + 1), :], 0.0)

            # gpsimd pad/dx work is emitted in deadline order so its FIFO
            # never delays a praw: mTr1 before batch 1's E-select, aT0/1
            # before phase5(0); the rest is injected into phase3(0) after
            # the first praw.
            for b in range(n_batch):
                phase1(b)
                phase2_dx(b)
                if b == 0:
                    phase2_praw(0, eng=nc.vector)
                    mtr_pad(1)
                if b == 1:
                    at_pad(0)
                    at_pad(1)

            def late_pads():
                mtr_pad(2)
                at_pad(2)
                at_pad(3)

            for it in range(1, n_batch + 1):
                b = it - 1
                injects = {}
                if it < n_batch:
                    # legal from r==0 on: the previous reader of this praw's
                    # mTr tile is phase3(it-2), already emitted
                    injects[0] = (lambda b2=it: phase2_praw(b2))
                if it == 1:
                    injects[3] = late_pads
                if it >= 2:
                    injects[1] = (lambda b5=it - 2: phase5(b5, chunk))
                ups = phase3(b, injects=injects)
                phase4(b, ups, chunk if b < n_batch - 1 else chunk // 2)
                if it == n_batch:
                    phase5(n_batch - 1, chunk // 2)

    nc.compile()
    return nc


def host_prep_shared(w1, b1, w2, b2, W1, bb1, W2, bb2):
    import ml_dtypes
    bf = ml_dtypes.bfloat16
    f = np.float32

    cwm = np.zeros((128, CW_COLS), f)
    cwm[:, CW_W1:CW_W1 + 2 * HID] = (
        np.asarray(w1, f).reshape(2, 128, HID).transpose(1, 0, 2).reshape(128, -1))
    cwm[0:HID, CW_W2:CW_W2 + C] = np.asarray(w2, f)
    # 0.5 midpoint factor folded into W1_sig
    cwm[:, CW_W1M:CW_W1M + KT * HID] = (
        0.5 * np.asarray(W1[C:], f).reshape(KT, 128, HID)
        .transpose(1, 0, 2).reshape(128, -1))
    cwm[0:C, CW_W1DX:CW_W1DX + HID] = np.asarray(W1[:C], f)
    for r in range(KT):
        for q in range(128):
            cwm[4 * r + q // 32, CW_E + 128 * r + q] = 1.0
    cwm[0:HID, CW_W2B:CW_W2B + DOUT] = np.asarray(W2, f)

    cbm = np.zeros((128, 4), f)
    cbm[0:HID, 0] = np.asarray(b1, f)
    cbm[0:C, 1] = np.asarray(b2, f)
    cbm[0:HID, 2] = np.asarray(bb1, f)
    cbm[:, 3] = np.asarray(bb2, f)

    return {
        "cw": cwm.astype(bf),
        "cb": cbm,
    }


_NC_CACHE = {}


def _get_nc():
    key = "full"
    if key not in _NC_CACHE:
        _NC_CACHE[key] = build_nc()
    return _NC_CACHE[key]


def kernel(x, w1, b1, w2, b2, W1, bb1, W2, bb2):
    global LAST_EXEC_NS, LAST_PROFILE, LAST_TRACE_PATH
    import ml_dtypes
    bf = ml_dtypes.bfloat16
    nc = _get_nc()
    shared = host_prep_shared(w1, b1, w2, b2, W1, bb1, W2, bb2)
    xbf = np.ascontiguousarray(x, np.float32).astype(bf)
    n_batch = T // L
    in_maps = []
    for core in range(N_CORES):
        xc = xbf[core * B_CORE:(core + 1) * B_CORE].reshape(T, DIN)
        # (256, T) -> (n_batch, 128, 2, L): [b][p][k][t] = xT[k*128+p, b*L+t]
        xT = xc.T.reshape(2, 128, n_batch, L)
        xTb = np.ascontiguousarray(xT.transpose(2, 1, 0, 3))
        m = dict(shared)
        m["xTb"] = np.ascontiguousarray(xTb)
        in_maps.append(m)
    # rare transient device flakes can surface as NaN output; retry those
    for attempt in range(3):
        try:
            res = bass_utils.run_bass_kernel_spmd(
                nc, in_maps, core_ids=list(range(N_CORES)), trace=TRACE,
            )
        except Exception:
            if not TRACE:
                raise
            res = bass_utils.run_bass_kernel_spmd(
                nc, in_maps, core_ids=list(range(N_CORES)), trace=False,
            )
        LAST_EXEC_NS = res.exec_time_ns
        LAST_PROFILE = res.profile_json
        LAST_TRACE_PATH = (res.instructions_and_trace or (None, None))[1]
        outs = [np.ascontiguousarray(
                    np.asarray(res.results[i]["out"], np.float32).T)
                .reshape(B_CORE, L, DOUT) for i in range(N_CORES)]
        full = np.concatenate(outs, axis=0)
        if np.isfinite(full).all():
            return full
    return full
